# revision 2
# baseline (speedup 1.0000x reference)
"""Trainium2 Bass kernel for nn_FRAMES_VisionTransformer_28166395527587.

The reference computation (drop CLS token -> 1D nearest resize 768->729 ->
reverse-patching reshape to (144,126,126) -> 3D nearest resize to (64,64,64))
is a pure gather with compile-time-constant index maps:

    out[b, 0, z, y, x] = hs[b, 1 + 196*(z//4) + 14*r + p, f[81*d0 + 9*d1 + d2]]

with  d0 = [0,2,4,6][z%4], i = z//4, c(y) = floor32(63y/32) = 9r + d1,
      c(x) = 9p + d2, f = float32-exact floor(arange(729) * 768/729).

This version is tuned for the DMA roofline (the kernel is pure data movement):

  * The only features ever referenced live in 4 contiguous windows of the
    768-wide feature dim: [0,85) u [170,255) u [341,426) u [511,597)
    (the per-d0 resize windows; 341 of 768 columns).  Host-side sharding
    slices those columns out (uniform contiguous column slices, no
    reordering) and casts to bf16, so each token row shrinks from 3072 B to
    a 704 B padded row.  bf16 quantization has rel-err <= 2^-9 ~ 2e-3,
    well inside the 2e-2 gate.
  * Token rows are then CONTIGUOUS in DRAM, so each load DMA moves whole
    14-token row-groups as single ~9.9 KB descriptors at full DMA-engine
    rate.  (The f32 baseline moved 288-352 B descriptors, which the DMA
    engines process at ~half rate; that made loads 2.5x slower per byte.)
  * All 14 row-group loads are issued up-front (both h-halves resident in
    SBUF simultaneously); compute rounds consume them in order.
  * Output is produced and stored as bf16 (identical values to the f32
    store of bf16-quantized inputs) and widened to f32 on the host.

Per (q, h) round: compaction copies fold the y-resize row-selection and the
feature-map run-compaction into M [32 yl, 126 c], then 6 strided copies
gather x into O [32 yl, 64 x], stored as one contiguous run per partition.

Sharding: pure data parallel, 8 batch samples per core.  CLS stripped
host-side so the 128 SBUF partitions hold the 128 (sample, couple) blocks.
"""

import numpy as np

# ---------------------------------------------------------------- constants
B_FULL = 64
N_CORES = 8
B_CORE = B_FULL // N_CORES  # 8 samples per core


def _nearest_f32(out_size, in_size):
    """float32-exact emulation of the reference's jnp _nearest_idx.

    jax computes floor(arange(out) * (in/out)) in float32; at j=486 the
    product rounds to 511.999... so floor gives 511, not the exact 512."""
    ratio = np.float32(in_size / out_size)
    j = np.arange(out_size, dtype=np.int32).astype(np.float32)
    return np.floor((j * ratio).astype(np.float32)).astype(np.int64)


_f = _nearest_f32(729, 768)  # feature resize map
DZ = [0, 2, 4, 6]  # d0 values for z%4
LOS = [int(_f[81 * d0]) for d0 in DZ]  # [0, 170, 341, 511]
WID = [int(_f[81 * d0 + 80]) - lo + 1 for d0, lo in zip(DZ, LOS)]  # [85,85,85,86]
WOFF = [0, WID[0], WID[0] + WID[1], WID[0] + WID[1] + WID[2]]  # [0,85,170,255]
U_ROW = 352  # union width 341, padded to a whole number of 32B beats


def _runs(vals):
    """Contiguous runs of an int sequence: [(start_idx, length)]."""
    runs, s = [], 0
    for k in range(1, len(vals)):
        if vals[k] != vals[k - 1] + 1:
            runs.append((s, k - s))
            s = k
    runs.append((s, len(vals) - s))
    return runs


def _compact_specs(q):
    """[(cls, yl0, d2_0, n, u0)]: copy token-row-class cls' feature run
    [u0, u0+n) (union-layout offsets) into M[yl, t, d2_0:+n].

    cls 'Z': physical row 0 of the h-half (single row, yl0 exact);
    'A': rows 2,4,6 (3 rows, yl = yl0 + 9j); 'B': rows 1,3,5 (3 rows)."""
    g = (_f[81 * DZ[q] + np.arange(81)] - LOS[q]).astype(int)
    woff = WOFF[q]
    specs = []
    for d1 in (1, 3, 5, 7):  # odd d1 -> row 0 (Z) + rows 2,4,6 (A)
        for (s, n) in _runs(g[9 * d1 : 9 * d1 + 9]):
            u0 = woff + int(g[9 * d1 + s])
            specs.append(("Z", (d1 + 1) // 2, s, n, u0))
            specs.append(("A", (d1 + 1) // 2 + 9, s, n, u0))
    for d1 in (2, 4, 6, 8):  # even d1 -> rows 1,3,5 (B)
        for (s, n) in _runs(g[9 * d1 : 9 * d1 + 9]):
            specs.append(("B", 5 + d1 // 2, s, n, woff + int(g[9 * d1 + s])))
    for (s, n) in _runs(g[0:9]):  # d1 = 0 main -> rows 1,3,5 (c = 9, 27, 45)
        specs.append(("B", 5, s, n, woff + int(g[s])))
    for (s, n) in _runs(g[0:9]):  # d1 = 0 special -> row 0 (c = 0 / 63)
        specs.append(("Z", 0, s, n, woff + int(g[s])))
    return specs


CSPECS = [_compact_specs(q) for q in range(4)]

# x-gather families: O[yl, ob+9g+okoff+k] = M[yl, ib+18g+ikoff+2k], k < nx
# (base shifts keep every rearrange-window inside the real 64/126 extents)
XFAM = [
    (1, 4, 0, 4, 1, 0),
    (5, 3, 0, 5, 9, 0),
    (28, 4, 5, 4, 54, 10),
    (37, 3, 0, 5, 72, 0),
]
XSINGLE = [(0, 0), (32, 63)]  # (x, c) singletons

# ------------------------------------------------------------- bass program
_NC_CACHE = None


def _build_nc():
    import concourse.bacc as bacc
    import concourse.tile as tile
    from concourse import mybir

    nc = bacc.Bacc(None, target_bir_lowering=False, debug=False)
    bf16 = mybir.dt.bfloat16

    # CLS stripped + feature dim sliced to the 4 accessed windows host-side.
    hs = nc.dram_tensor("hs", (B_CORE, 3136, U_ROW), bf16, kind="ExternalInput")
    out = nc.dram_tensor("out", (B_CORE, 1, 64, 64, 64), bf16, kind="ExternalOutput")

    # [(b i), r, (p u)]: token rows are contiguous -> (p u) merges into one
    # 14*352-elem contiguous span per (partition, row): single ~9.9KB DMA
    # descriptors.
    hs_r = hs.ap().rearrange("b (i r p) u -> (b i) r (p u)", i=16, r=14, p=14)
    # [(b i), q, (y x)]: z = 4i+q; y,x merge into one contiguous dim
    out_v = out.ap().rearrange("b c (i q) y x -> (b i) c q (y x)", i=16, q=4)

    with tile.TileContext(nc) as tc:
        with (
            tc.tile_pool(name="lp", bufs=1) as lp,
            tc.tile_pool(name="mp", bufs=3) as mp,
            tc.tile_pool(name="op", bufs=3) as op,
        ):
            def cp(eng, dst, src):
                if eng is nc.scalar:
                    eng.copy(out=dst, in_=src)
                else:
                    eng.tensor_copy(out=dst, in_=src)

            # ---- all loads issued up-front; both h-halves stay resident
            S = {}
            for h in (0, 1):
                t = lp.tile([128, 7 * 14 * U_ROW], bf16, tag=f"H{h}")
                t3 = t.rearrange("p (k x) -> p k x", k=7)
                for k in range(7):
                    nc.sync.dma_start(out=t3[:, k], in_=hs_r[:, 7 * h + k, :])
                S[h] = t.rearrange("p (k t u) -> p k t u", k=7, u=U_ROW)

            rnd = 0
            for h in range(2):
                S4 = S[h]
                for q in range(4):
                    specs = CSPECS[q]
                    # final rounds: finer sub-rounds shorten the kernel tail
                    if rnd < 6:
                        subs = [(0, 32)]
                    elif rnd == 6:
                        subs = [(0, 14), (14, 32)]
                    else:
                        subs = [(0, 8), (8, 16), (16, 24), (24, 32)]
                    # compaction and x-gather alternate engines per round
                    ce = nc.scalar if rnd % 2 == 0 else nc.vector
                    xe = nc.vector if rnd % 2 == 0 else nc.scalar
                    for (ylr0, ylr1) in subs:
                        nyl = ylr1 - ylr0
                        # ---- compact + y-select -> M [nyl yl, 126 c]
                        M = mp.tile([128, nyl * 126], bf16, tag="M")
                        M4 = M.rearrange("p (yl t d2) -> p yl t d2", yl=nyl, d2=9)
                        for (cls, yl0, d2_0, n, u0) in specs:
                            if cls == "Z":
                                if not (ylr0 <= yl0 < ylr1):
                                    continue
                                y0 = yl0 - ylr0
                                dst = M4[:, y0 : y0 + 1, :, d2_0 : d2_0 + n]
                                src = S4[:, 0:1, :, u0 : u0 + n]
                            else:
                                k0 = 2 if cls == "A" else 1
                                ks = [j for j in range(3) if ylr0 <= yl0 + 9 * j < ylr1]
                                if not ks:
                                    continue
                                j0, j1 = ks[0], ks[-1] + 1
                                y0 = yl0 + 9 * j0 - ylr0
                                dst = M4[
                                    :, y0 : y0 + 9 * (j1 - j0 - 1) + 1 : 9,
                                    :, d2_0 : d2_0 + n,
                                ]
                                src = S4[:, k0 + 2 * j0 : k0 + 2 * (j1 - 1) + 1 : 2,
                                         :, u0 : u0 + n]
                            cp(ce, dst, src)

                        # ---- x-gather -> O [nyl yl, 64 x]
                        O = op.tile([128, nyl * 64], bf16, tag="O")
                        O3 = O.rearrange("p (yl x) -> p yl x", x=64)
                        M3 = M.rearrange("p (yl c) -> p yl c", c=126)
                        for (x, c) in XSINGLE:
                            cp(xe, O3[:, :, x : x + 1], M3[:, :, c : c + 1])
                        for (ob, og, okoff, nx, ib, ikoff) in XFAM:
                            o4 = O3[:, :, ob : ob + 9 * og].rearrange(
                                "p yl (g k) -> p yl g k", g=og
                            )[:, :, :, okoff : okoff + nx]
                            i4 = M3[:, :, ib : ib + 18 * og].rearrange(
                                "p yl (g c) -> p yl g c", g=og
                            )[:, :, :, ikoff : ikoff + 2 * nx - 1 : 2]
                            cp(xe, o4, i4)

                        # ---- store: contiguous run per partition
                        ob0 = 2048 * h + 64 * ylr0
                        nc.scalar.dma_start(
                            out=out_v[:, 0, q, ob0 : ob0 + 64 * nyl],
                            in_=O[:, :],
                        )
                    rnd += 1

    nc.compile()
    return nc


def _get_nc():
    global _NC_CACHE
    if _NC_CACHE is None:
        _NC_CACHE = _build_nc()
    return _NC_CACHE


# ------------------------------------------------------------------ runner
def _in_maps(hidden_states: np.ndarray) -> list:
    import ml_dtypes

    hs = np.asarray(hidden_states, dtype=np.float32)
    assert hs.shape == (B_FULL, 3137, 768), hs.shape
    maps = []
    for c in range(N_CORES):
        blk = hs[c * B_CORE : (c + 1) * B_CORE, 1:, :]
        u = np.zeros((B_CORE, 3136, U_ROW), dtype=ml_dtypes.bfloat16)
        for lo, w, off in zip(LOS, WID, WOFF):
            u[:, :, off : off + w] = blk[:, :, lo : lo + w]
        maps.append({"hs": u})
    return maps


def kernel(hidden_states: np.ndarray) -> np.ndarray:
    import time

    from concourse import bass_utils

    nc = _get_nc()
    in_maps = _in_maps(hidden_states)
    last_err = None
    for attempt in range(3):
        try:
            res = bass_utils.run_bass_kernel_spmd(
                nc, in_maps, core_ids=list(range(N_CORES))
            )
            return np.concatenate(
                [np.asarray(r["out"]).astype(np.float32) for r in res.results],
                axis=0,
            )
        except Exception as e:  # transient device hiccups self-heal in ~1 min
            last_err = e
            time.sleep(45 * (attempt + 1))
    raise last_err


# revision 3
# speedup vs baseline: 1.0530x; 1.0530x over previous
"""Trainium2 Bass kernel for nn_FRAMES_VisionTransformer_28166395527587.

The reference computation (drop CLS token -> 1D nearest resize 768->729 ->
reverse-patching reshape to (144,126,126) -> 3D nearest resize to (64,64,64))
is a pure gather with compile-time-constant index maps:

    out[b, 0, z, y, x] = hs[b, 1 + 196*(z//4) + 14*r + p, f[81*d0 + 9*d1 + d2]]

with  d0 = [0,2,4,6][z%4], i = z//4, c(y) = floor32(63y/32) = 9r + d1,
      c(x) = 9p + d2, f = float32-exact floor(arange(729) * 768/729).

Tuned for the DMA roofline (the kernel is pure data movement):

  * Only 4 contiguous windows of the 768-wide feature dim are ever
    referenced: [0,85) u [170,255) u [341,426) u [511,597) (341 of 768
    columns).  Host-side sharding slices those columns out (uniform
    contiguous column slices, no reordering) and casts to bf16; each token
    row shrinks from 3072 B to a 704 B padded row.  bf16 quantization has
    rel-err <= 2^-9 ~ 2e-3, well inside the 2e-2 gate.
  * Token rows are then CONTIGUOUS in DRAM: each load DMA moves whole
    14-token row-groups as single ~9.9 KB descriptors at full DMA-engine
    rate (f32 baseline moved 288-352 B descriptors at ~half rate).
  * All loads are issued up-front (both h-halves resident in SBUF).  The
    second half is split into two row-group tiles (rows 0-2 / rows 3-6) and
    its rounds into two yl-blocks, so late-round compute starts as soon as
    its rows land instead of waiting for the whole half.
  * Fixed engine roles: DVE does all compaction copies (bf16 2x rate,
    cheapest per instruction; Z+A row-classes merged into single 4-row
    strided copies), ACT does the x-gather, the sync ring issues all DMA.
  * Output is produced and stored as bf16 (identical values to an f32
    store of bf16-quantized inputs) and widened to f32 on the host.

Sharding: pure data parallel, 8 batch samples per core.  CLS stripped
host-side so the 128 SBUF partitions hold the 128 (sample, couple) blocks.
"""

import numpy as np

# ---------------------------------------------------------------- constants
B_FULL = 64
N_CORES = 8
B_CORE = B_FULL // N_CORES  # 8 samples per core


def _nearest_f32(out_size, in_size):
    """float32-exact emulation of the reference's jnp _nearest_idx.

    jax computes floor(arange(out) * (in/out)) in float32; at j=486 the
    product rounds to 511.999... so floor gives 511, not the exact 512."""
    ratio = np.float32(in_size / out_size)
    j = np.arange(out_size, dtype=np.int32).astype(np.float32)
    return np.floor((j * ratio).astype(np.float32)).astype(np.int64)


_f = _nearest_f32(729, 768)  # feature resize map
DZ = [0, 2, 4, 6]  # d0 values for z%4
LOS = [int(_f[81 * d0]) for d0 in DZ]  # [0, 170, 341, 511]
WID = [int(_f[81 * d0 + 80]) - lo + 1 for d0, lo in zip(DZ, LOS)]  # [85,85,85,86]
WOFF = [0, WID[0], WID[0] + WID[1], WID[0] + WID[1] + WID[2]]  # [0,85,170,255]
U_ROW = 352  # union width 341, padded to a whole number of 32B beats


def _runs(vals):
    """Contiguous runs of an int sequence: [(start_idx, length)]."""
    runs, s = [], 0
    for k in range(1, len(vals)):
        if vals[k] != vals[k - 1] + 1:
            runs.append((s, k - s))
            s = k
    runs.append((s, len(vals) - s))
    return runs


def _specs(q):
    """Per-q compaction copy specs in union-layout offsets.

    za: [(v, s, n, u0)]  yl = v + 9j (j=0..3)  <- physical row 2j   (Z+A)
    b3: [(yl0, s, n, u0)] yl = yl0 + 9j (j=0..2) <- physical row 2j+1 (B)
    z0: [(s, n, u0)]     yl = 0                 <- physical row 0
    """
    g = (_f[81 * DZ[q] + np.arange(81)] - LOS[q]).astype(int)
    woff = WOFF[q]
    za, b3, z0 = [], [], []
    for d1 in (1, 3, 5, 7):  # odd d1 -> rows 0,2,4,6 (Z merged with A)
        for (s, n) in _runs(g[9 * d1 : 9 * d1 + 9]):
            za.append(((d1 + 1) // 2, s, n, woff + int(g[9 * d1 + s])))
    for d1 in (2, 4, 6, 8):  # even d1 -> rows 1,3,5
        for (s, n) in _runs(g[9 * d1 : 9 * d1 + 9]):
            b3.append((5 + d1 // 2, s, n, woff + int(g[9 * d1 + s])))
    for (s, n) in _runs(g[0:9]):  # d1 = 0 main -> rows 1,3,5 (c = 9, 27, 45)
        b3.append((5, s, n, woff + int(g[s])))
    for (s, n) in _runs(g[0:9]):  # d1 = 0 special -> row 0 (c = 0 / 63)
        z0.append((s, n, woff + int(g[s])))
    return za, b3, z0


SPECS = [_specs(q) for q in range(4)]

# x-gather families: O[yl, ob+9g+okoff+k] = M[yl, ib+18g+ikoff+2k], k < nx
# (base shifts keep every rearrange-window inside the real 64/126 extents)
XFAM = [
    (1, 4, 0, 4, 1, 0),
    (5, 3, 0, 5, 9, 0),
    (28, 4, 5, 4, 54, 10),
    (37, 3, 0, 5, 72, 0),
]
XSINGLE = [(0, 0), (32, 63)]  # (x, c) singletons

# ------------------------------------------------------------- bass program
_NC_CACHE = None


def _build_nc():
    import concourse.bacc as bacc
    import concourse.tile as tile
    from concourse import mybir

    nc = bacc.Bacc(None, target_bir_lowering=False, debug=False)
    bf16 = mybir.dt.bfloat16

    # CLS stripped + feature dim sliced to the 4 accessed windows host-side.
    hs = nc.dram_tensor("hs", (B_CORE, 3136, U_ROW), bf16, kind="ExternalInput")
    out = nc.dram_tensor("out", (B_CORE, 1, 64, 64, 64), bf16, kind="ExternalOutput")

    # [(b i), r, (p u)]: token rows are contiguous -> (p u) merges into one
    # 14*352-elem contiguous span per (partition, row): ~9.9KB descriptors.
    hs_r = hs.ap().rearrange("b (i r p) u -> (b i) r (p u)", i=16, r=14, p=14)
    # [(b i), q, (y x)]: z = 4i+q; y,x merge into one contiguous dim
    out_v = out.ap().rearrange("b c (i q) y x -> (b i) c q (y x)", i=16, q=4)

    with tile.TileContext(nc) as tc:
        with (
            tc.tile_pool(name="lp", bufs=1) as lp,
            tc.tile_pool(name="mp", bufs=3) as mp,
            tc.tile_pool(name="op", bufs=3) as op,
        ):
            # ---- all loads issued up-front on the sync HWDGE ring.
            # h=0: one 7-row tile.  h=1: rows 0-2 and rows 3-6 tiles, so the
            # yl-blocks of the late rounds unblock as soon as their rows land.
            def load_rows(tag, rows):
                t = lp.tile([128, len(rows) * 14 * U_ROW], bf16, tag=tag)
                t3 = t.rearrange("p (k x) -> p k x", k=len(rows))
                for j, r in enumerate(rows):
                    nc.sync.dma_start(out=t3[:, j], in_=hs_r[:, r, :])
                return t.rearrange("p (k t u) -> p k t u", k=len(rows), u=U_ROW)

            S7 = load_rows("H0", [0, 1, 2, 3, 4, 5, 6])
            SA = load_rows("H1A", [7, 8, 9])
            SB = load_rows("H1B", [10, 11, 12, 13])

            # rowmaps: physical row (0..6 within the half) -> (view, local k)
            RM0 = {k: (S7, k) for k in range(7)}
            RM1 = {0: (SA, 0), 1: (SA, 1), 2: (SA, 2),
                   3: (SB, 0), 4: (SB, 1), 5: (SB, 2), 6: (SB, 3)}

            ce, xe = nc.vector, nc.scalar  # DVE compaction, ACT x-gather

            def emit(M4, ylr0, js, yl_of, row_of, rowmap, s, n, u0):
                """One strided copy per maximal consecutive-j run that stays
                inside a single source tile (local row stride is always 2)."""
                while js:
                    v, lk = rowmap[row_of(js[0])]
                    e = 1
                    while (
                        e < len(js)
                        and js[e] == js[e - 1] + 1
                        and rowmap[row_of(js[e])][0] is v
                    ):
                        e += 1
                    run, js = js[:e], js[e:]
                    y0 = yl_of(run[0]) - ylr0
                    m = len(run)
                    dst = M4[:, y0 : y0 + 9 * (m - 1) + 1 : 9, :, s : s + n]
                    src = v[:, lk : lk + 2 * (m - 1) + 1 : 2, :, u0 : u0 + n]
                    ce.tensor_copy(out=dst, in_=src)

            def compact(M4, ylr0, ylr1, q, rowmap):
                za, b3, z0 = SPECS[q]
                for (v, s, n, u0) in za:
                    js = [j for j in range(4) if ylr0 <= v + 9 * j < ylr1]
                    emit(M4, ylr0, js, lambda j: v + 9 * j, lambda j: 2 * j,
                         rowmap, s, n, u0)
                for (yl0, s, n, u0) in b3:
                    js = [j for j in range(3) if ylr0 <= yl0 + 9 * j < ylr1]
                    emit(M4, ylr0, js, lambda j: yl0 + 9 * j,
                         lambda j: 2 * j + 1, rowmap, s, n, u0)
                if ylr0 == 0:
                    v, lk = rowmap[0]
                    for (s, n, u0) in z0:
                        ce.tensor_copy(
                            out=M4[:, 0:1, :, s : s + n],
                            in_=v[:, lk : lk + 1, :, u0 : u0 + n],
                        )

            for h in range(2):
                rowmap = RM0 if h == 0 else RM1
                for q in range(4):
                    rnd = 4 * h + q
                    if h == 0:
                        blocks = [(0, 32)]
                    elif rnd < 7:
                        blocks = [(0, 14), (14, 32)]
                    else:  # finer tail
                        blocks = [(0, 14), (14, 23), (23, 32)]
                    for (ylr0, ylr1) in blocks:
                        nyl = ylr1 - ylr0
                        # ---- compact + y-select -> M [nyl yl, 126 c]
                        M = mp.tile([128, nyl * 126], bf16, tag="M")
                        M4 = M.rearrange("p (yl t d2) -> p yl t d2",
                                         yl=nyl, d2=9)
                        compact(M4, ylr0, ylr1, q, rowmap)

                        # ---- x-gather -> O [nyl yl, 64 x]
                        O = op.tile([128, nyl * 64], bf16, tag="O")
                        O3 = O.rearrange("p (yl x) -> p yl x", x=64)
                        M3 = M.rearrange("p (yl c) -> p yl c", c=126)
                        for (x, c) in XSINGLE:
                            xe.copy(out=O3[:, :, x : x + 1],
                                    in_=M3[:, :, c : c + 1])
                        for (ob, og, okoff, nx, ib, ikoff) in XFAM:
                            o4 = O3[:, :, ob : ob + 9 * og].rearrange(
                                "p yl (g k) -> p yl g k", g=og
                            )[:, :, :, okoff : okoff + nx]
                            i4 = M3[:, :, ib : ib + 18 * og].rearrange(
                                "p yl (g c) -> p yl g c", g=og
                            )[:, :, :, ikoff : ikoff + 2 * nx - 1 : 2]
                            xe.copy(out=o4, in_=i4)

                        # ---- store: contiguous run per partition
                        ob0 = 2048 * h + 64 * ylr0
                        nc.sync.dma_start(
                            out=out_v[:, 0, q, ob0 : ob0 + 64 * nyl],
                            in_=O[:, :],
                        )

    nc.compile()
    return nc


def _get_nc():
    global _NC_CACHE
    if _NC_CACHE is None:
        _NC_CACHE = _build_nc()
    return _NC_CACHE


# ------------------------------------------------------------------ runner
def _in_maps(hidden_states: np.ndarray) -> list:
    import ml_dtypes

    hs = np.asarray(hidden_states, dtype=np.float32)
    assert hs.shape == (B_FULL, 3137, 768), hs.shape
    maps = []
    for c in range(N_CORES):
        blk = hs[c * B_CORE : (c + 1) * B_CORE, 1:, :]
        u = np.zeros((B_CORE, 3136, U_ROW), dtype=ml_dtypes.bfloat16)
        for lo, w, off in zip(LOS, WID, WOFF):
            u[:, :, off : off + w] = blk[:, :, lo : lo + w]
        maps.append({"hs": u})
    return maps


def kernel(hidden_states: np.ndarray) -> np.ndarray:
    import time

    from concourse import bass_utils

    nc = _get_nc()
    in_maps = _in_maps(hidden_states)
    last_err = None
    for attempt in range(3):
        try:
            res = bass_utils.run_bass_kernel_spmd(
                nc, in_maps, core_ids=list(range(N_CORES))
            )
            return np.concatenate(
                [np.asarray(r["out"]).astype(np.float32) for r in res.results],
                axis=0,
            )
        except Exception as e:  # transient device hiccups self-heal in ~1 min
            last_err = e
            time.sleep(45 * (attempt + 1))
    raise last_err
